# revision 7
# baseline (speedup 1.0000x reference)
"""Embedding lookup (gather + scale) on 8 TRN2 NeuronCores.

Strategy: data-parallel over tokens. The [50257, 1024] f32 table is
preprocessed on the host into bf16 with the sqrt(d_model)=32 scale folded in
(exact: x32 is a pure exponent shift in bf16, so bf16(32*W) == 32*bf16(W)
bit-for-bit; quantization rel err ~3e-3, well under the 2e-2 gate) and
replicated to every core's DRAM. The 8*2048 = 16384 tokens are split into 8
chunks of 2048. Each core gathers its 2048 rows with 16 indirect DMAs (128
rows / 256KB each), and streams each [128, 1024] bf16 tile back out with a
contiguous HWDGE store as soon as its gather lands. The host widens the bf16
output to f32, which is numerically exact (pure mantissa pad), so the
returned output is bit-identical to an on-device upcast. No collectives.

Built directly on raw bass (no TileContext) with per-lane DMA semaphores:
gather j increments lane j%8 by 16; the matching store waits for the lane's
FULL cumulative count (16*(j//8+1)), which is rigorous under SDMA engine
skew because the target equals the lane's maximum possible value at that
point. Performance is bound by the indirect-gather descriptor wall: 2048
random 2KB HBM reads processed serially per SDMA engine (~193ns each, 16
engines) ~= 24.7us, plus ~9us fixed NEFF preamble (engine start, library
loads, entry barrier) and ~4us drain postamble.

HW-validated constraints baked in (CoreSim accepts all of these, HW does
not -- do not "simplify"):
- indirect_dma_start offset AP must be a single column ([128, 1]); an
  offset AP with k>1 columns makes the DGE fetch k CONSECUTIVE rows from
  the FIRST index (and a 3D dest slice crashes the exec unit).
- The gather dest must lower to a plain 2D AP: flat column slices of a 2D
  SBUF tensor work; [128, 1, 1024] slices of a 3D tile land wrong.
- Cross-dtype DVE ops (bf16 in -> f32 out) produce garbage on HW.
"""

import math

import numpy as np

D_VOCAB = 50257
D_MODEL = 1024
N_CORES = 8
TOK_PER_CORE = 2048
P = 128
N_TILES = TOK_PER_CORE // P  # 16
SCALE = math.sqrt(D_MODEL)  # 32.0

_progs = {}


def _build_program():
    import concourse.bacc as bacc
    import concourse.mybir as mybir
    from concourse import bass
    from contextlib import ExitStack

    nc = bacc.Bacc("TRN2", debug=False, num_devices=N_CORES)
    tokens = nc.dram_tensor(
        "tokens", [TOK_PER_CORE], mybir.dt.int32, kind="ExternalInput"
    ).ap()
    w = nc.dram_tensor(
        "w", [D_VOCAB, D_MODEL], mybir.dt.bfloat16, kind="ExternalInput"
    ).ap()
    out = nc.dram_tensor(
        "out", [TOK_PER_CORE, D_MODEL], mybir.dt.bfloat16, kind="ExternalOutput"
    ).ap()

    with ExitStack() as ctx:
        idx_tile = ctx.enter_context(
            nc.sbuf_tensor("idx_tile", [P, N_TILES], mybir.dt.int32)
        )
        emb = ctx.enter_context(
            nc.sbuf_tensor("emb", [P, N_TILES * D_MODEL], mybir.dt.bfloat16)
        )
        idx_sem = ctx.enter_context(nc.semaphore(name="idx_sem"))
        gather_sems = [
            ctx.enter_context(nc.semaphore(name=f"gather_sem{i}")) for i in range(8)
        ]
        store_sem_s = ctx.enter_context(nc.semaphore(name="store_sem_s"))
        store_sem_a = ctx.enter_context(nc.semaphore(name="store_sem_a"))
        block = ctx.enter_context(nc.Block())

        def do_store(eng, j, store_sem):
            eng.wait_ge(gather_sems[j % 8], 16 * (j // 8 + 1))
            eng.dma_start(
                out=out[j * P : (j + 1) * P, :],
                in_=emb[:, j * D_MODEL : (j + 1) * D_MODEL],
            ).then_inc(store_sem, 16)

        # The host uploads tokens PRE-PERMUTED: tokens_in[p*16 + j] =
        # original_tokens[j*128 + p], so the idx load is one contiguous
        # 64B-per-partition DMA, gather j's column j holds the indices for
        # output rows j*128..(j+1)*128, and every store is a fully
        # contiguous 512KB block.
        tok_pj = tokens.rearrange("(p j) -> p j", p=P)

        @block.sync
        def _(sync):
            idx_dma = sync.dma_start(out=idx_tile[:], in_=tok_pj)
            idx_dma.then_inc(idx_sem, 16)
            nc._idx_dma_ins = idx_dma.ins
            for j in range(0, N_TILES, 2):
                do_store(sync, j, store_sem_s)
            sync.wait_ge(store_sem_s, 16 * (N_TILES // 2))

        @block.gpsimd
        def _(gpsimd):
            gpsimd.wait_ge(idx_sem, 16)
            for j in range(N_TILES):
                gpsimd.indirect_dma_start(
                    out=emb[:, j * D_MODEL : (j + 1) * D_MODEL],
                    out_offset=None,
                    in_=w[:],
                    in_offset=bass.IndirectOffsetOnAxis(
                        ap=idx_tile[:, j : j + 1], axis=0
                    ),
                ).then_inc(gather_sems[j % 8], 16)

        @block.scalar
        def _(scalar):
            for j in range(1, N_TILES, 2):
                do_store(scalar, j, store_sem_a)
            scalar.wait_ge(store_sem_a, 16 * (N_TILES // 2))

    # Hoist the idx DMA ahead of the SP entry-barrier wait: it has no
    # dependencies (the gathers wait on idx_sem), so it can issue while the
    # other engines finish their init. Best-effort: skip on any mismatch.
    try:
        import concourse.mybir as mybir

        entry = nc.main_func.blocks[0]
        sp_block = next(
            b for b in nc.main_func.blocks if nc._idx_dma_ins in b.instructions
        )
        barrier_pos = next(
            k
            for k, ins in enumerate(entry.instructions)
            if ins.engine == mybir.EngineType.SP and "barrier" in ins.name
        )
        sp_block.instructions.remove(nc._idx_dma_ins)
        entry.instructions.insert(barrier_pos, nc._idx_dma_ins)
    except (StopIteration, AttributeError):
        pass

    nc.compile()
    return nc


def _get_program():
    if "raw" not in _progs:
        _progs["raw"] = _build_program()
    return _progs["raw"]


def _to_bf16_scaled(a):
    """f32 table -> bf16(32 * table), round-to-nearest-even."""
    import ml_dtypes

    return (np.asarray(a, dtype=np.float32) * np.float32(SCALE)).astype(
        ml_dtypes.bfloat16
    )


def _run(tokens, W_E, trace=False, core_ids=None):
    from concourse.bass_utils import run_bass_kernel_spmd

    tokens = np.ascontiguousarray(np.asarray(tokens).astype(np.int32))
    assert tokens.size == N_CORES * TOK_PER_CORE
    flat = tokens.reshape(-1)
    w_bf = np.ascontiguousarray(_to_bf16_scaled(W_E))

    if core_ids is None:
        core_ids = list(range(N_CORES))
    nc = _get_program()
    in_maps = []
    for c in core_ids:
        chunk = flat[c * TOK_PER_CORE : (c + 1) * TOK_PER_CORE]
        # device expects tokens_in[p*16 + j] = chunk[j*128 + p]
        permuted = np.ascontiguousarray(chunk.reshape(N_TILES, P).T.reshape(-1))
        in_maps.append({"tokens": permuted, "w": w_bf})
    res = run_bass_kernel_spmd(nc, in_maps, core_ids=core_ids, trace=trace)
    # device output is bf16; widening to f32 is exact (pure mantissa pad)
    out = np.stack(
        [
            np.asarray(res.results[i]["out"], dtype=np.float32)
            for i in range(len(core_ids))
        ],
        axis=0,
    )
    return out.reshape(len(core_ids), TOK_PER_CORE, D_MODEL), res


def kernel(tokens, W_E):
    out, _ = _run(tokens, W_E, trace=False)
    return out


# revision 9
# speedup vs baseline: 1.0129x; 1.0129x over previous
"""Embedding lookup (gather + scale) on 8 TRN2 NeuronCores.

Strategy: data-parallel over tokens. The [50257, 1024] f32 table is
preprocessed on the host into bf16 with the sqrt(d_model)=32 scale folded in
(exact: x32 is a pure exponent shift in bf16, so bf16(32*W) == 32*bf16(W)
bit-for-bit; quantization rel err ~3e-3, well under the 2e-2 gate) and
replicated to every core's DRAM. The 8*2048 = 16384 tokens are split into 8
chunks of 2048. Each core gathers its 2048 rows with 16 indirect DMAs (128
rows / 256KB each), and streams each [128, 1024] bf16 tile back out with a
contiguous HWDGE store as soon as its gather lands. The host widens the bf16
output to f32, which is numerically exact (pure mantissa pad), so the
returned output is bit-identical to an on-device upcast. No collectives.

Built directly on raw bass (no TileContext) with per-lane DMA semaphores:
gather j increments lane j%8 by 16; the matching store waits for the lane's
FULL cumulative count (16*(j//8+1)), which is rigorous under SDMA engine
skew because the target equals the lane's maximum possible value at that
point. Performance is bound by the indirect-gather descriptor wall: 2048
random 2KB HBM reads processed serially per SDMA engine (~193ns each, 16
engines) ~= 24.7us, plus ~9us fixed NEFF preamble (engine start, library
loads, entry barrier) and ~4us drain postamble.

HW-validated constraints baked in (CoreSim accepts all of these, HW does
not -- do not "simplify"):
- indirect_dma_start offset AP must be a single column ([128, 1]); an
  offset AP with k>1 columns makes the DGE fetch k CONSECUTIVE rows from
  the FIRST index (and a 3D dest slice crashes the exec unit).
- The gather dest must lower to a plain 2D AP: flat column slices of a 2D
  SBUF tensor work; [128, 1, 1024] slices of a 3D tile land wrong.
- Cross-dtype DVE ops (bf16 in -> f32 out) produce garbage on HW.
"""

import math

import numpy as np

D_VOCAB = 50257
D_MODEL = 1024
N_CORES = 8
TOK_PER_CORE = 2048
P = 128
N_TILES = TOK_PER_CORE // P  # 16
SCALE = math.sqrt(D_MODEL)  # 32.0

_progs = {}


def _build_program():
    import concourse.bacc as bacc
    import concourse.mybir as mybir
    from concourse import bass
    from contextlib import ExitStack

    nc = bacc.Bacc("TRN2", debug=False, num_devices=N_CORES)
    tokens = nc.dram_tensor(
        "tokens", [TOK_PER_CORE], mybir.dt.int32, kind="ExternalInput"
    ).ap()
    w = nc.dram_tensor(
        "w", [D_VOCAB, D_MODEL], mybir.dt.bfloat16, kind="ExternalInput"
    ).ap()
    out = nc.dram_tensor(
        "out", [TOK_PER_CORE, D_MODEL], mybir.dt.bfloat16, kind="ExternalOutput"
    ).ap()

    with ExitStack() as ctx:
        idx_tile = ctx.enter_context(
            nc.sbuf_tensor("idx_tile", [P, N_TILES], mybir.dt.int32)
        )
        emb = ctx.enter_context(
            nc.sbuf_tensor("emb", [P, N_TILES * D_MODEL], mybir.dt.bfloat16)
        )
        idx_sem = ctx.enter_context(nc.semaphore(name="idx_sem"))
        gather_sems = [
            ctx.enter_context(nc.semaphore(name=f"gather_sem{i}")) for i in range(8)
        ]
        store_sem_s = ctx.enter_context(nc.semaphore(name="store_sem_s"))
        store_sem_a = ctx.enter_context(nc.semaphore(name="store_sem_a"))
        block = ctx.enter_context(nc.Block())

        def do_store(eng, j, store_sem):
            eng.wait_ge(gather_sems[j % 8], 16 * (j // 8 + 1))
            eng.dma_start(
                out=out[j * P : (j + 1) * P, :],
                in_=emb[:, j * D_MODEL : (j + 1) * D_MODEL],
            ).then_inc(store_sem, 16)

        # The host uploads tokens PRE-PERMUTED: tokens_in[p*16 + j] =
        # original_tokens[j*128 + p], so the idx load is one contiguous
        # 64B-per-partition DMA, gather j's column j holds the indices for
        # output rows j*128..(j+1)*128, and every store is a fully
        # contiguous 512KB block.
        tok_pj = tokens.rearrange("(p j) -> p j", p=P)

        @block.sync
        def _(sync):
            for j in range(0, N_TILES, 2):
                do_store(sync, j, store_sem_s)
            sync.wait_ge(store_sem_s, 16 * (N_TILES // 2))

        @block.gpsimd
        def _(gpsimd):
            idx_dma = gpsimd.dma_start(out=idx_tile[:], in_=tok_pj)
            idx_dma.then_inc(idx_sem, 16)
            nc._idx_dma_ins = idx_dma.ins
            gpsimd.wait_ge(idx_sem, 16)
            for j in range(N_TILES):
                gpsimd.indirect_dma_start(
                    out=emb[:, j * D_MODEL : (j + 1) * D_MODEL],
                    out_offset=None,
                    in_=w[:],
                    in_offset=bass.IndirectOffsetOnAxis(
                        ap=idx_tile[:, j : j + 1], axis=0
                    ),
                ).then_inc(gather_sems[j % 8], 16)

        @block.scalar
        def _(scalar):
            for j in range(1, N_TILES, 2):
                do_store(scalar, j, store_sem_a)
            scalar.wait_ge(store_sem_a, 16 * (N_TILES // 2))

    # Hoist the idx DMA (Pool SWDGE) ahead of the entry barrier, at the
    # const-memset position: it has no dependencies (the gathers wait on
    # idx_sem), so it completes during the fixed init sequence and the
    # first gather is not gated by a post-barrier load. Best-effort: skip
    # on any mismatch.
    try:
        entry = nc.main_func.blocks[0]
        pool_block = next(
            b for b in nc.main_func.blocks if nc._idx_dma_ins in b.instructions
        )
        memset_pos = next(
            k
            for k, ins in enumerate(entry.instructions)
            if type(ins).__name__ == "InstMemset"
        )
        pool_block.instructions.remove(nc._idx_dma_ins)
        entry.instructions.insert(memset_pos, nc._idx_dma_ins)
    except (StopIteration, AttributeError):
        pass

    nc.compile()
    return nc


def _get_program():
    if "raw" not in _progs:
        _progs["raw"] = _build_program()
    return _progs["raw"]


def _to_bf16_scaled(a):
    """f32 table -> bf16(32 * table), round-to-nearest-even."""
    import ml_dtypes

    return (np.asarray(a, dtype=np.float32) * np.float32(SCALE)).astype(
        ml_dtypes.bfloat16
    )


def _run(tokens, W_E, trace=False, core_ids=None):
    from concourse.bass_utils import run_bass_kernel_spmd

    tokens = np.ascontiguousarray(np.asarray(tokens).astype(np.int32))
    assert tokens.size == N_CORES * TOK_PER_CORE
    flat = tokens.reshape(-1)
    w_bf = np.ascontiguousarray(_to_bf16_scaled(W_E))

    if core_ids is None:
        core_ids = list(range(N_CORES))
    nc = _get_program()
    in_maps = []
    for c in core_ids:
        chunk = flat[c * TOK_PER_CORE : (c + 1) * TOK_PER_CORE]
        # device expects tokens_in[p*16 + j] = chunk[j*128 + p]
        permuted = np.ascontiguousarray(chunk.reshape(N_TILES, P).T.reshape(-1))
        in_maps.append({"tokens": permuted, "w": w_bf})
    res = run_bass_kernel_spmd(nc, in_maps, core_ids=core_ids, trace=trace)
    # device output is bf16; widening to f32 is exact (pure mantissa pad)
    out = np.stack(
        [
            np.asarray(res.results[i]["out"], dtype=np.float32)
            for i in range(len(core_ids))
        ],
        axis=0,
    )
    return out.reshape(len(core_ids), TOK_PER_CORE, D_MODEL), res


def kernel(tokens, W_E):
    out, _ = _run(tokens, W_E, trace=False)
    return out
